# revision 30
# baseline (speedup 1.0000x reference)
"""Causal multi-head attention on 8 TRN2 NeuronCores.

Problem: x[4, 2048, 768], 12 heads x d_head 64, causal softmax attention.

Sharding: core c handles batch b = c//2 and the 6-head group h0 = 6*(c%2).
Each core computes its partial output o_partial[2048, 768] = sum over its 6
heads of (softmax(QK^T/8) V) @ W_O.  The two cores sharing a batch are summed
on the host (part of unsharding), so the device graph needs no collectives.

Device layout (per core) avoids every on-chip transpose:
  - host passes x^T  [768, 2048]  (xt)
  - Q^T, K^T [384, 2048] = W^T @ x^T   (lhsT = W slice, rhs = x^T)
  - V        [2048, 384]  = x @ W_V    (lhsT = x^T slice, rhs = W_V)
  - scores^T blocks [128k, 512q] = (K^T slice).T @ Q^T slice  (per head, K=64)
  - exp on ACT (scale=1/8 folded in); causal mask = 0/1 multiply on diagonal
    blocks; V is stored augmented with a ones column per head so the single
    AV matmul produces both z^T rows (64) and the softmax denominators (row 64)
  - z^T normalized via DMA-broadcast reciprocal row, stored as ZT [384, 2048]
  - out = (ZT).T @ W_O   (lhsT = ZT slice, rhs = W_O)
All matmuls run in bf16 (1 cycle/column vs 4 for fp32; f32 PSUM accum).
"""

import sys

if "/opt/trn_rl_repo" not in sys.path:
    sys.path.insert(0, "/opt/trn_rl_repo")

import numpy as np
import ml_dtypes

BF16NP = ml_dtypes.bfloat16


def _ensure_ntff_hook():
    """The agent image's `antenv` lacks `axon_hooks`, which bass_utils needs
    for trace=True under axon. Recreate it via sys.modules injection using the
    boot helper's ctypes wrapper around libaxon_pjrt.so."""
    import types
    if "antenv.axon_hooks" in sys.modules:
        return
    try:
        from trn_agent_boot.trn_boot import _ntff_profile_via_ctypes
        hook = _ntff_profile_via_ctypes("/opt/axon/libaxon_pjrt.so")
    except Exception:
        hook = None
    m = types.ModuleType("antenv.axon_hooks")
    m._hook = hook
    m.get_axon_ntff_profile_hook = lambda: m._hook
    def _set(h):
        m._hook = h
    m.set_axon_ntff_profile_hook = _set
    sys.modules["antenv.axon_hooks"] = m


_ensure_ntff_hook()

import concourse.bass as bass
import concourse.tile as tile
from concourse import bacc, mybir
from concourse.bass_utils import run_bass_kernel_spmd

F32 = mybir.dt.float32
BF16 = mybir.dt.bfloat16
AF = mybir.ActivationFunctionType

D = 768          # d_model
S = 2048         # seq
E = 64           # d_head
NHC = 6          # heads per core
HE = NHC * E     # 384
KD = D // 128    # 6 k-chunks over d_model
NQS = S // 512   # 4 q-supertiles
NKB = S // 128   # 16 k-blocks
B = 4

LAST_EXEC_TIME_NS = None
_GRAPH_CACHE = {}


def _build_graph(qkv_bias: bool) -> bass.Bass:
    nc = bacc.Bacc("TRN2", target_bir_lowering=False)
    xt = nc.declare_dram_parameter("xt", [D, S], BF16, isOutput=False)
    wq = nc.declare_dram_parameter("wq", [D, HE], BF16, isOutput=False)
    wk = nc.declare_dram_parameter("wk", [D, HE], BF16, isOutput=False)
    wv = nc.declare_dram_parameter("wv", [D, HE], BF16, isOutput=False)
    wo = nc.declare_dram_parameter("wo", [HE, D], BF16, isOutput=False)
    mask = nc.declare_dram_parameter("mask", [128, 128], BF16, isOutput=False)
    if qkv_bias:
        bq = nc.declare_dram_parameter("bq", [HE, 1], F32, isOutput=False)
        bk = nc.declare_dram_parameter("bk", [HE, 1], F32, isOutput=False)
        bv = nc.declare_dram_parameter("bv", [1, HE], BF16, isOutput=False)
    out = nc.declare_dram_parameter("out", [S, D], F32, isOutput=True)

    with tile.TileContext(nc) as tc:
        with tc.tile_pool(name="persist", bufs=1) as persist:
            QT = [persist.tile([128, S], BF16, tag=f"qt{m}", name=f"qt{m}") for m in range(3)]
            KT = [persist.tile([128, S], BF16, tag=f"kt{m}", name=f"kt{m}") for m in range(3)]
            ZT = [persist.tile([128, S], BF16, tag=f"zt{m}", name=f"zt{m}") for m in range(3)]
            VA = [persist.tile([128, NHC * 65], BF16, tag=f"va{s}", name=f"va{s}") for s in range(16)]
            WO = [persist.tile([128, D], BF16, tag=f"wo{m}", name=f"wo{m}") for m in range(3)]
            MSK = persist.tile([128, 128], BF16, tag="mask", name="mask_sb")
            for m in range(3):
                nc.sync.dma_start(out=WO[m][:], in_=wo[m * 128:(m + 1) * 128, :])
            nc.sync.dma_start(out=MSK[:], in_=mask[:])
            ONES = persist.tile([1, 128], BF16, tag="ones", name="ones_sb")
            nc.vector.memset(ONES[:], 1.0)
            if qkv_bias:
                BQ = persist.tile([128, 3], F32, tag="bq", name="bq_sb")
                BK = persist.tile([128, 3], F32, tag="bk", name="bk_sb")
                BV = persist.tile([1, HE], BF16, tag="bv", name="bv_sb")
                for m in range(3):
                    nc.sync.dma_start(out=BQ[:, m:m + 1], in_=bq[m * 128:(m + 1) * 128, :])
                    nc.sync.dma_start(out=BK[:, m:m + 1], in_=bk[m * 128:(m + 1) * 128, :])
                nc.sync.dma_start(out=BV[:], in_=bv[:])

            # ---------------- phase B/C: projections ----------------
            with tc.tile_pool(name="loadA", bufs=1) as loadA, \
                 tc.tile_pool(name="psA", bufs=2, space="PSUM") as psA:
                XT = [loadA.tile([128, S], BF16, tag=f"xt{k}", name=f"xt{k}") for k in range(KD)]
                WQs = [loadA.tile([128, HE], BF16, tag=f"wq{k}", name=f"wq{k}") for k in range(KD)]
                WKs = [loadA.tile([128, HE], BF16, tag=f"wk{k}", name=f"wk{k}") for k in range(KD)]
                WVs = [loadA.tile([128, HE], BF16, tag=f"wv{k}", name=f"wv{k}") for k in range(KD)]
                for k in range(KD):
                    nc.sync.dma_start(out=XT[k][:], in_=xt[k * 128:(k + 1) * 128, :])
                    nc.sync.dma_start(out=WQs[k][:], in_=wq[k * 128:(k + 1) * 128, :])
                    nc.sync.dma_start(out=WKs[k][:], in_=wk[k * 128:(k + 1) * 128, :])
                    nc.sync.dma_start(out=WVs[k][:], in_=wv[k * 128:(k + 1) * 128, :])

                # Q^T and K^T: [384, 2048] = W.T @ x^T
                for Wt, Ot, Bt in ((WQs, QT, "bq"), (WKs, KT, "bk")):
                    for m in range(3):
                        for n in range(NQS):
                            ps = psA.tile([128, 512], F32, tag="psA", name="ps_proj")
                            for k in range(KD):
                                nc.tensor.matmul(
                                    ps[:],
                                    Wt[k][:, m * 128:(m + 1) * 128],
                                    XT[k][:, n * 512:(n + 1) * 512],
                                    start=(k == 0), stop=(k == KD - 1))
                            dst = Ot[m][:, n * 512:(n + 1) * 512]
                            if qkv_bias:
                                bias_t = BQ if Bt == "bq" else BK
                                nc.scalar.activation(dst, ps[:], AF.Copy,
                                                     bias=bias_t[:, m:m + 1])
                            else:
                                nc.vector.tensor_copy(dst, ps[:])

                # V (augmented with ones column per head): VA[sc] = [128, 6*65]
                for sc in range(16):
                    nc.vector.memset(VA[sc][:], 1.0)
                    ps = psA.tile([128, HE], F32, tag="psV", name="ps_v", bufs=3)
                    for k in range(KD):
                        nc.tensor.matmul(
                            ps[:],
                            XT[k][:, sc * 128:(sc + 1) * 128],
                            WVs[k][:],
                            start=(k == 0), stop=False if qkv_bias else (k == KD - 1))
                    if qkv_bias:
                        nc.tensor.matmul(
                            ps[:], ONES[:], BV[:],
                            start=False, stop=True)
                    nc.vector.tensor_copy(
                        VA[sc][:].rearrange("p (h c) -> p h c", c=65)[:, :, 0:64],
                        ps[:].rearrange("p (h c) -> p h c", c=64))

            # ---------------- phase D: attention ----------------
            # Per head, k-block-outer: each KT/VA block is loaded onto the PE
            # once and streamed against every q-supertile that needs it.
            # psz[t] accumulators finish at j=4t+3 and normalize immediately
            # (staggered), so PSUM frees progressively.
            with tc.tile_pool(name="workE", bufs=20) as workE, \
                 tc.tile_pool(name="workZ", bufs=8) as workZ, \
                 tc.tile_pool(name="work2", bufs=3) as work2, \
                 tc.tile_pool(name="dramP", bufs=3, space="DRAM") as dramP, \
                 tc.tile_pool(name="psS", bufs=2, space="PSUM") as psS, \
                 tc.tile_pool(name="psZ", bufs=1, space="PSUM") as psZ:
                pending = []   # deferred normalize closures (keep PE fed)
                av_q = []      # aged AV work: (av_fn, norm_fn | None)
                AV_LAG = 14    # items of et ageing (AV inputs always ready)

                def drain_pending(upto):
                    while len(pending) > upto:
                        pending.pop(0)()

                def pump_avs(lag):
                    while len(av_q) > lag:
                        av_fn, norm_fn = av_q.pop(0)
                        av_fn()
                        if norm_fn is not None:
                            norm_fn()
                            drain_pending(1)

                # Heads are processed in PAIRS: the even head lives on PE rows
                # 0-63, the odd head on rows 64-127.  Emitting the two score
                # matmuls adjacently makes them run CONCURRENTLY (disjoint row
                # groups) and keeps the PE array fully active, which is what
                # holds the HAM clock at 2.4 GHz (a pure K=64 stream never
                # leaves 1.2 GHz).  q-supertiles go in halves of two so the
                # psz accumulators (2 heads x 2 supers) fit in 4 PSUM banks.
                for hp in range(3):
                    for thalf in range(2):
                        ts = (2 * thalf, 2 * thalf + 1)
                        psz = {}  # key (par, t); allocated lazily at first AV

                        def emit_normalize(par, t, hp=hp, psz=psz):
                            ho = par * 64
                            # drain psz out of PSUM fast (frees the bank)
                            zraw = workZ.tile([65, 512], BF16, tag="zraw", name="zraw")
                            nc.vector.tensor_copy(zraw[:], psz[(par, t)][:])

                            # reciprocal of the denominator row, reshaped across
                            # all 128 DVE lanes via a DRAM bounce ([1,512] ->
                            # [128,4]); a 1-lane [1,512] reciprocal costs 3.3us.
                            dd = dramP.tile([1, 512], BF16, tag="dd", name="dd")
                            nc.sync.dma_start(out=dd[:], in_=zraw[64:65, :])
                            dd_ap = dd[:]
                            rp = work2.tile([128, 4], BF16, tag="rp", name="rp")
                            nc.sync.dma_start(out=rp[:], in_=bass.AP(
                                tensor=dd_ap.tensor, offset=dd_ap.offset,
                                ap=[[4, 128], [1, 4]]))
                            rcp = work2.tile([128, 4], BF16, tag="rcp", name="rcp")
                            with nc.allow_low_precision(reason="softmax recip bf16"):
                                nc.vector.reciprocal(rcp[:], rp[:])
                            rcd = dramP.tile([1, 512], BF16, tag="rcd", name="rcd")
                            rcd_ap = rcd[:]
                            nc.sync.dma_start(out=bass.AP(
                                tensor=rcd_ap.tensor, offset=rcd_ap.offset,
                                ap=[[4, 128], [1, 4]]), in_=rcp[:])
                            bc = work2.tile([64, 512], BF16, tag="bc", name="bc")
                            nc.sync.dma_start(out=bc[:], in_=bass.AP(
                                tensor=rcd_ap.tensor, offset=rcd_ap.offset,
                                ap=[[0, 64], rcd_ap.ap[-1]]))

                            def part2():
                                # final scale, deferred so DMA latency is hidden
                                nc.vector.tensor_mul(
                                    ZT[hp][ho:ho + 64, t * 512:(t + 1) * 512],
                                    zraw[0:64, :], bc[:])
                            pending.append(part2)

                        def emit_s_pair(j, t, hp=hp, psz=psz,
                                        emit_normalize=emit_normalize):
                            r = j - 4 * t  # >= 0 only on the diagonal
                            q0 = 128 * r if r >= 0 else 0
                            # both heads' scores in ONE 2-bank psum tile: a
                            # single slot allocation keeps the two matmuls
                            # adjacent in the PE stream, so they run
                            # concurrently (disjoint row groups) and hold the
                            # clock warm
                            pss = psS.tile([128, 1024], F32, tag="pss",
                                           name="pss")
                            for par in (0, 1):
                                ho = par * 64
                                nc.tensor.matmul(
                                    pss[:, par * 512 + q0:par * 512 + 512],
                                    KT[hp][ho:ho + 64, j * 128:(j + 1) * 128],
                                    QT[hp][ho:ho + 64, t * 512 + q0:(t + 1) * 512],
                                    start=True, stop=True)
                            et = workE.tile([128, 1024], BF16, tag="et",
                                            name="et")
                            if r >= 0:
                                # diagonal: two exps (the gap is unwritten psum)
                                for par in (0, 1):
                                    nc.scalar.activation(
                                        et[:, par * 512 + q0:par * 512 + 512],
                                        pss[:, par * 512 + q0:par * 512 + 512],
                                        AF.Exp, scale=0.125)
                                    nc.vector.tensor_mul(
                                        et[:, par * 512 + q0:par * 512 + q0 + 128],
                                        et[:, par * 512 + q0:par * 512 + q0 + 128],
                                        MSK[:])
                            else:
                                nc.scalar.activation(et[:], pss[:],
                                                     AF.Exp, scale=0.125)
                            for par in (0, 1):
                                def av_fn(par=par, et=et, j=j, t=t, q0=q0):
                                    if j == 0:
                                        psz[(par, t)] = psZ.tile(
                                            [65, 512], F32,
                                            tag=f"psz{par}{t % 2}",
                                            name=f"psz{par}{t % 2}")
                                    h = 2 * hp + par
                                    nc.tensor.matmul(
                                        psz[(par, t)][:, q0:512],
                                        VA[j][:, h * 65:(h + 1) * 65],
                                        et[:, par * 512 + q0:par * 512 + 512],
                                        start=(j == 0), stop=(j == 4 * t + 3))
                                norm_fn = (
                                    lambda par=par, t=t, f=emit_normalize:
                                    f(par, t)) if j == 4 * t + 3 else None
                                av_q.append((av_fn, norm_fn))

                        for j in range(4 * (ts[-1] + 1)):
                            for t in ts:
                                if t >= j // 4:
                                    emit_s_pair(j, t)
                                    pump_avs(AV_LAG)
                pump_avs(0)
                drain_pending(0)

            # ---------------- phase E: output projection ----------------
            with tc.tile_pool(name="workO", bufs=4) as workO, \
                 tc.tile_pool(name="psO", bufs=2, space="PSUM") as psO:
                for mc in range(16):
                    for half, tag in ((0, "poA"), (1, "poB")):
                        po = psO.tile([128, HE], F32, tag=tag, name=tag)
                        n0 = half * HE
                        for k in range(3):
                            nc.tensor.matmul(
                                po[:],
                                ZT[k][:, mc * 128:(mc + 1) * 128],
                                WO[k][:, n0:n0 + HE],
                                start=(k == 0), stop=(k == 2))
                        ob = workO.tile([128, HE], F32, tag=f"ob{half}", name=f"ob{half}")
                        nc.vector.tensor_copy(ob[:], po[:])
                        nc.sync.dma_start(
                            out=out[mc * 128:(mc + 1) * 128, n0:n0 + HE],
                            in_=ob[:])
    nc.compile()
    return nc


def _build_mask() -> np.ndarray:
    # triangle for the strict-diagonal 128x128 strip: 1.0 iff q_local >= k_local
    kl = np.arange(128)[:, None]
    ql = np.arange(128)[None, :]
    return (ql >= kl).astype(np.float32)


def kernel(**inputs) -> np.ndarray:
    global LAST_EXEC_TIME_NS
    x = np.asarray(inputs["normalized_resid_pre"], dtype=np.float32)
    W_Q = np.asarray(inputs["W_Q"], dtype=np.float32)
    W_K = np.asarray(inputs["W_K"], dtype=np.float32)
    W_V = np.asarray(inputs["W_V"], dtype=np.float32)
    W_O = np.asarray(inputs["W_O"], dtype=np.float32)
    b_Q = np.asarray(inputs["b_Q"], dtype=np.float32)
    b_K = np.asarray(inputs["b_K"], dtype=np.float32)
    b_V = np.asarray(inputs["b_V"], dtype=np.float32)
    b_O = np.asarray(inputs["b_O"], dtype=np.float32)

    qkv_bias = bool(b_Q.any() or b_K.any() or b_V.any())
    key = qkv_bias
    if key not in _GRAPH_CACHE:
        _GRAPH_CACHE[key] = _build_graph(qkv_bias)
    nc = _GRAPH_CACHE[key]

    mask = _build_mask()
    in_maps = []
    for c in range(8):
        b, h0 = c // 2, NHC * (c % 2)
        im = {
            "xt": np.ascontiguousarray(x[b].T).astype(BF16NP),
            "wq": np.ascontiguousarray(
                W_Q[h0:h0 + NHC].transpose(1, 0, 2).reshape(D, HE)).astype(BF16NP),
            "wk": np.ascontiguousarray(
                W_K[h0:h0 + NHC].transpose(1, 0, 2).reshape(D, HE)).astype(BF16NP),
            "wv": np.ascontiguousarray(
                W_V[h0:h0 + NHC].transpose(1, 0, 2).reshape(D, HE)).astype(BF16NP),
            "wo": np.ascontiguousarray(W_O[h0:h0 + NHC].reshape(HE, D)).astype(BF16NP),
            "mask": mask.astype(BF16NP),
        }
        if qkv_bias:
            im["bq"] = np.ascontiguousarray(b_Q[h0:h0 + NHC].reshape(HE, 1))
            im["bk"] = np.ascontiguousarray(b_K[h0:h0 + NHC].reshape(HE, 1))
            im["bv"] = np.ascontiguousarray(b_V[h0:h0 + NHC].reshape(1, HE)).astype(BF16NP)
        in_maps.append(im)

    import os
    trace = bool(os.environ.get("KERNEL_TRACE"))
    res = run_bass_kernel_spmd(nc, in_maps, core_ids=list(range(8)), trace=trace)
    LAST_EXEC_TIME_NS = res.exec_time_ns
    results = res.results

    out = np.empty((B, S, D), dtype=np.float32)
    for b in range(B):
        out[b] = results[2 * b]["out"] + results[2 * b + 1]["out"]
    if b_O.any():
        out += b_O
    return out


# revision 32
# speedup vs baseline: 1.0548x; 1.0548x over previous
"""Causal multi-head attention on 8 TRN2 NeuronCores.

Problem: x[4, 2048, 768], 12 heads x d_head 64, causal softmax attention.

Sharding: core c handles batch b = c//2 and the 6-head group h0 = 6*(c%2).
Each core computes its partial output o_partial[2048, 768] = sum over its 6
heads of (softmax(QK^T/8) V) @ W_O.  The two cores sharing a batch are summed
on the host (part of unsharding), so the device graph needs no collectives.

Device layout (per core) avoids every on-chip transpose:
  - host passes x^T  [768, 2048]  (xt)
  - Q^T, K^T [384, 2048] = W^T @ x^T   (lhsT = W slice, rhs = x^T)
  - V        [2048, 384]  = x @ W_V    (lhsT = x^T slice, rhs = W_V)
  - scores^T blocks [128k, 512q] = (K^T slice).T @ Q^T slice  (per head, K=64)
  - exp on ACT (scale=1/8 folded in); causal mask = 0/1 multiply on diagonal
    blocks; V is stored augmented with a ones column per head so the single
    AV matmul produces both z^T rows (64) and the softmax denominators (row 64)
  - z^T normalized via DMA-broadcast reciprocal row, stored as ZT [384, 2048]
  - out = (ZT).T @ W_O   (lhsT = ZT slice, rhs = W_O)
All matmuls run in bf16 (1 cycle/column vs 4 for fp32; f32 PSUM accum).
"""

import sys

if "/opt/trn_rl_repo" not in sys.path:
    sys.path.insert(0, "/opt/trn_rl_repo")

import numpy as np
import ml_dtypes

BF16NP = ml_dtypes.bfloat16


def _ensure_ntff_hook():
    """The agent image's `antenv` lacks `axon_hooks`, which bass_utils needs
    for trace=True under axon. Recreate it via sys.modules injection using the
    boot helper's ctypes wrapper around libaxon_pjrt.so."""
    import types
    if "antenv.axon_hooks" in sys.modules:
        return
    try:
        from trn_agent_boot.trn_boot import _ntff_profile_via_ctypes
        hook = _ntff_profile_via_ctypes("/opt/axon/libaxon_pjrt.so")
    except Exception:
        hook = None
    m = types.ModuleType("antenv.axon_hooks")
    m._hook = hook
    m.get_axon_ntff_profile_hook = lambda: m._hook
    def _set(h):
        m._hook = h
    m.set_axon_ntff_profile_hook = _set
    sys.modules["antenv.axon_hooks"] = m


_ensure_ntff_hook()

import concourse.bass as bass
import concourse.tile as tile
from concourse import bacc, mybir
from concourse.bass_utils import run_bass_kernel_spmd

F32 = mybir.dt.float32
BF16 = mybir.dt.bfloat16
AF = mybir.ActivationFunctionType

D = 768          # d_model
S = 2048         # seq
E = 64           # d_head
NHC = 6          # heads per core
HE = NHC * E     # 384
KD = D // 128    # 6 k-chunks over d_model
NQS = S // 512   # 4 q-supertiles
NKB = S // 128   # 16 k-blocks
B = 4

LAST_EXEC_TIME_NS = None
_GRAPH_CACHE = {}


def _build_graph(qkv_bias: bool) -> bass.Bass:
    nc = bacc.Bacc("TRN2", target_bir_lowering=False)
    xt = nc.declare_dram_parameter("xt", [D, S], BF16, isOutput=False)
    wq = nc.declare_dram_parameter("wq", [D, HE], BF16, isOutput=False)
    wk = nc.declare_dram_parameter("wk", [D, HE], BF16, isOutput=False)
    wv = nc.declare_dram_parameter("wv", [D, HE], BF16, isOutput=False)
    wo = nc.declare_dram_parameter("wo", [HE, D], BF16, isOutput=False)
    mask = nc.declare_dram_parameter("mask", [128, 128], BF16, isOutput=False)
    if qkv_bias:
        bq = nc.declare_dram_parameter("bq", [HE, 1], F32, isOutput=False)
        bk = nc.declare_dram_parameter("bk", [HE, 1], F32, isOutput=False)
        bv = nc.declare_dram_parameter("bv", [1, HE], BF16, isOutput=False)
    out = nc.declare_dram_parameter("out", [S, D], F32, isOutput=True)

    with tile.TileContext(nc) as tc:
        with tc.tile_pool(name="persist", bufs=1) as persist:
            QT = [persist.tile([128, S], BF16, tag=f"qt{m}", name=f"qt{m}") for m in range(3)]
            KT = [persist.tile([128, S], BF16, tag=f"kt{m}", name=f"kt{m}") for m in range(3)]
            ZT = [persist.tile([128, S], BF16, tag=f"zt{m}", name=f"zt{m}") for m in range(3)]
            VA = [persist.tile([128, NHC * 65], BF16, tag=f"va{s}", name=f"va{s}") for s in range(16)]
            WO = [persist.tile([128, D], BF16, tag=f"wo{m}", name=f"wo{m}") for m in range(3)]
            MSK = persist.tile([128, 128], BF16, tag="mask", name="mask_sb")
            for m in range(3):
                nc.sync.dma_start(out=WO[m][:], in_=wo[m * 128:(m + 1) * 128, :])
            nc.sync.dma_start(out=MSK[:], in_=mask[:])
            ONES = persist.tile([1, 128], BF16, tag="ones", name="ones_sb")
            nc.vector.memset(ONES[:], 1.0)
            if qkv_bias:
                BQ = persist.tile([128, 3], F32, tag="bq", name="bq_sb")
                BK = persist.tile([128, 3], F32, tag="bk", name="bk_sb")
                BV = persist.tile([1, HE], BF16, tag="bv", name="bv_sb")
                for m in range(3):
                    nc.sync.dma_start(out=BQ[:, m:m + 1], in_=bq[m * 128:(m + 1) * 128, :])
                    nc.sync.dma_start(out=BK[:, m:m + 1], in_=bk[m * 128:(m + 1) * 128, :])
                nc.sync.dma_start(out=BV[:], in_=bv[:])

            # ---------------- phase B/C: projections ----------------
            with tc.tile_pool(name="loadA", bufs=1) as loadA, \
                 tc.tile_pool(name="psA", bufs=3, space="PSUM") as psA:
                XT = [loadA.tile([128, S], BF16, tag=f"xt{k}", name=f"xt{k}") for k in range(KD)]
                WQs = [loadA.tile([128, HE], BF16, tag=f"wq{k}", name=f"wq{k}") for k in range(KD)]
                WKs = [loadA.tile([128, HE], BF16, tag=f"wk{k}", name=f"wk{k}") for k in range(KD)]
                WVs = [loadA.tile([128, HE], BF16, tag=f"wv{k}", name=f"wv{k}") for k in range(KD)]
                for k in range(KD):
                    nc.sync.dma_start(out=XT[k][:], in_=xt[k * 128:(k + 1) * 128, :])
                    nc.sync.dma_start(out=WQs[k][:], in_=wq[k * 128:(k + 1) * 128, :])
                    nc.sync.dma_start(out=WKs[k][:], in_=wk[k * 128:(k + 1) * 128, :])
                    nc.sync.dma_start(out=WVs[k][:], in_=wv[k * 128:(k + 1) * 128, :])

                # Q^T and K^T: [384, 2048] = W.T @ x^T
                for Wt, Ot, Bt in ((WQs, QT, "bq"), (WKs, KT, "bk")):
                    for m in range(3):
                        for n in range(NQS):
                            ps = psA.tile([128, 512], F32, tag="psA", name="ps_proj")
                            for k in range(KD):
                                nc.tensor.matmul(
                                    ps[:],
                                    Wt[k][:, m * 128:(m + 1) * 128],
                                    XT[k][:, n * 512:(n + 1) * 512],
                                    start=(k == 0), stop=(k == KD - 1))
                            dst = Ot[m][:, n * 512:(n + 1) * 512]
                            if qkv_bias:
                                bias_t = BQ if Bt == "bq" else BK
                                nc.scalar.activation(dst, ps[:], AF.Copy,
                                                     bias=bias_t[:, m:m + 1])
                            else:
                                nc.vector.tensor_copy(dst, ps[:])

                # V (augmented with ones column per head): VA[sc] = [128, 6*65]
                for sc in range(16):
                    nc.vector.memset(VA[sc][:], 1.0)
                    ps = psA.tile([128, HE], F32, tag="psV", name="ps_v", bufs=3)
                    for k in range(KD):
                        nc.tensor.matmul(
                            ps[:],
                            XT[k][:, sc * 128:(sc + 1) * 128],
                            WVs[k][:],
                            start=(k == 0), stop=False if qkv_bias else (k == KD - 1))
                    if qkv_bias:
                        nc.tensor.matmul(
                            ps[:], ONES[:], BV[:],
                            start=False, stop=True)
                    nc.vector.tensor_copy(
                        VA[sc][:].rearrange("p (h c) -> p h c", c=65)[:, :, 0:64],
                        ps[:].rearrange("p (h c) -> p h c", c=64))

            # ---------------- phase D: attention ----------------
            # Per head, k-block-outer: each KT/VA block is loaded onto the PE
            # once and streamed against every q-supertile that needs it.
            # psz[t] accumulators finish at j=4t+3 and normalize immediately
            # (staggered), so PSUM frees progressively.
            with tc.tile_pool(name="workE", bufs=16) as workE, \
                 tc.tile_pool(name="workZ", bufs=12) as workZ, \
                 tc.tile_pool(name="work2", bufs=4) as work2, \
                 tc.tile_pool(name="dramP", bufs=3, space="DRAM") as dramP, \
                 tc.tile_pool(name="psS", bufs=2, space="PSUM") as psS, \
                 tc.tile_pool(name="psZ", bufs=1, space="PSUM") as psZ:
                pending = []   # deferred normalize closures (keep PE fed)
                av_q = []      # aged AV work: (av_fn, norm_fn | None)
                AV_LAG = 10    # items (~3 k-block groups) of et ageing

                def drain_pending(upto):
                    while len(pending) > upto:
                        pending.pop(0)()

                def pump_avs(lag):
                    while len(av_q) > lag:
                        av_fn, norm_fn = av_q.pop(0)
                        av_fn()
                        if norm_fn is not None:
                            norm_fn()
                            drain_pending(1)

                # Heads are processed in PAIRS: the even head lives on PE rows
                # 0-63, the odd head on rows 64-127.  Emitting the two score
                # matmuls adjacently makes them run CONCURRENTLY (disjoint row
                # groups) and keeps the PE array fully active, which is what
                # holds the HAM clock at 2.4 GHz (a pure K=64 stream never
                # leaves 1.2 GHz).  q-supertiles go in halves of two so the
                # psz accumulators (2 heads x 2 supers) fit in 4 PSUM banks.
                for hp in range(3):
                    for thalf in range(2):
                        ts = (2 * thalf, 2 * thalf + 1)
                        psz = {}  # key (par, t); allocated lazily at first AV

                        def emit_normalize(par, t, hp=hp, psz=psz):
                            ho = par * 64
                            # drain psz out of PSUM fast (frees the bank)
                            zraw = workZ.tile([65, 512], BF16, tag="zraw", name="zraw")
                            nc.vector.tensor_copy(zraw[:], psz[(par, t)][:])

                            # reciprocal of the denominator row, reshaped across
                            # all 128 DVE lanes via a DRAM bounce ([1,512] ->
                            # [128,4]); a 1-lane [1,512] reciprocal costs 3.3us.
                            dd = dramP.tile([1, 512], BF16, tag="dd", name="dd")
                            nc.sync.dma_start(out=dd[:], in_=zraw[64:65, :])
                            dd_ap = dd[:]
                            rp = work2.tile([128, 4], BF16, tag="rp", name="rp")
                            nc.sync.dma_start(out=rp[:], in_=bass.AP(
                                tensor=dd_ap.tensor, offset=dd_ap.offset,
                                ap=[[4, 128], [1, 4]]))
                            rcp = work2.tile([128, 4], BF16, tag="rcp", name="rcp")
                            with nc.allow_low_precision(reason="softmax recip bf16"):
                                nc.vector.reciprocal(rcp[:], rp[:])
                            rcd = dramP.tile([1, 512], BF16, tag="rcd", name="rcd")
                            rcd_ap = rcd[:]
                            nc.sync.dma_start(out=bass.AP(
                                tensor=rcd_ap.tensor, offset=rcd_ap.offset,
                                ap=[[4, 128], [1, 4]]), in_=rcp[:])
                            bc = work2.tile([64, 512], BF16, tag="bc", name="bc")
                            nc.sync.dma_start(out=bc[:], in_=bass.AP(
                                tensor=rcd_ap.tensor, offset=rcd_ap.offset,
                                ap=[[0, 64], rcd_ap.ap[-1]]))

                            def part2():
                                # final scale, deferred so DMA latency is hidden
                                nc.vector.tensor_mul(
                                    ZT[hp][ho:ho + 64, t * 512:(t + 1) * 512],
                                    zraw[0:64, :], bc[:])
                            pending.append(part2)

                        def emit_s_pair(j, t, hp=hp, psz=psz,
                                        emit_normalize=emit_normalize):
                            r = j - 4 * t  # >= 0 only on the diagonal
                            q0 = 128 * r if r >= 0 else 0
                            # both heads' scores in ONE 2-bank psum tile: a
                            # single slot allocation keeps the two matmuls
                            # adjacent in the PE stream, so they run
                            # concurrently (disjoint row groups) and hold the
                            # clock warm
                            pss = psS.tile([128, 1024], F32, tag="pss",
                                           name="pss")
                            for par in (0, 1):
                                ho = par * 64
                                nc.tensor.matmul(
                                    pss[:, par * 512 + q0:par * 512 + 512],
                                    KT[hp][ho:ho + 64, j * 128:(j + 1) * 128],
                                    QT[hp][ho:ho + 64, t * 512 + q0:(t + 1) * 512],
                                    start=True, stop=True)
                            et = workE.tile([128, 1024], BF16, tag="et",
                                            name="et")
                            if r >= 0:
                                # diagonal: two exps (the gap is unwritten psum)
                                for par in (0, 1):
                                    nc.scalar.activation(
                                        et[:, par * 512 + q0:par * 512 + 512],
                                        pss[:, par * 512 + q0:par * 512 + 512],
                                        AF.Exp, scale=0.125)
                                    nc.vector.tensor_mul(
                                        et[:, par * 512 + q0:par * 512 + q0 + 128],
                                        et[:, par * 512 + q0:par * 512 + q0 + 128],
                                        MSK[:])
                            else:
                                nc.scalar.activation(et[:], pss[:],
                                                     AF.Exp, scale=0.125)
                            for par in (0, 1):
                                def av_fn(par=par, et=et, j=j, t=t, q0=q0):
                                    if j == 0:
                                        psz[(par, t)] = psZ.tile(
                                            [65, 512], F32,
                                            tag=f"psz{par}{t % 2}",
                                            name=f"psz{par}{t % 2}")
                                    h = 2 * hp + par
                                    nc.tensor.matmul(
                                        psz[(par, t)][:, q0:512],
                                        VA[j][:, h * 65:(h + 1) * 65],
                                        et[:, par * 512 + q0:par * 512 + 512],
                                        start=(j == 0), stop=(j == 4 * t + 3))
                                norm_fn = (
                                    lambda par=par, t=t, f=emit_normalize:
                                    f(par, t)) if j == 4 * t + 3 else None
                                av_q.append((av_fn, norm_fn))

                        for j in range(4 * (ts[-1] + 1)):
                            for t in ts:
                                if t >= j // 4:
                                    emit_s_pair(j, t)
                                    pump_avs(AV_LAG)
                pump_avs(0)
                drain_pending(0)

            # ---------------- phase E: output projection ----------------
            with tc.tile_pool(name="workO", bufs=4) as workO, \
                 tc.tile_pool(name="psO", bufs=2, space="PSUM") as psO:
                for mc in range(16):
                    for half, tag in ((0, "poA"), (1, "poB")):
                        po = psO.tile([128, HE], F32, tag=tag, name=tag)
                        n0 = half * HE
                        for k in range(3):
                            nc.tensor.matmul(
                                po[:],
                                ZT[k][:, mc * 128:(mc + 1) * 128],
                                WO[k][:, n0:n0 + HE],
                                start=(k == 0), stop=(k == 2))
                        ob = workO.tile([128, HE], F32, tag=f"ob{half}", name=f"ob{half}")
                        nc.vector.tensor_copy(ob[:], po[:])
                        nc.sync.dma_start(
                            out=out[mc * 128:(mc + 1) * 128, n0:n0 + HE],
                            in_=ob[:])
    nc.compile()
    return nc


def _build_mask() -> np.ndarray:
    # triangle for the strict-diagonal 128x128 strip: 1.0 iff q_local >= k_local
    kl = np.arange(128)[:, None]
    ql = np.arange(128)[None, :]
    return (ql >= kl).astype(np.float32)


def kernel(**inputs) -> np.ndarray:
    global LAST_EXEC_TIME_NS
    x = np.asarray(inputs["normalized_resid_pre"], dtype=np.float32)
    W_Q = np.asarray(inputs["W_Q"], dtype=np.float32)
    W_K = np.asarray(inputs["W_K"], dtype=np.float32)
    W_V = np.asarray(inputs["W_V"], dtype=np.float32)
    W_O = np.asarray(inputs["W_O"], dtype=np.float32)
    b_Q = np.asarray(inputs["b_Q"], dtype=np.float32)
    b_K = np.asarray(inputs["b_K"], dtype=np.float32)
    b_V = np.asarray(inputs["b_V"], dtype=np.float32)
    b_O = np.asarray(inputs["b_O"], dtype=np.float32)

    qkv_bias = bool(b_Q.any() or b_K.any() or b_V.any())
    key = qkv_bias
    if key not in _GRAPH_CACHE:
        _GRAPH_CACHE[key] = _build_graph(qkv_bias)
    nc = _GRAPH_CACHE[key]

    mask = _build_mask()
    in_maps = []
    for c in range(8):
        b, h0 = c // 2, NHC * (c % 2)
        im = {
            "xt": np.ascontiguousarray(x[b].T).astype(BF16NP),
            "wq": np.ascontiguousarray(
                W_Q[h0:h0 + NHC].transpose(1, 0, 2).reshape(D, HE)).astype(BF16NP),
            "wk": np.ascontiguousarray(
                W_K[h0:h0 + NHC].transpose(1, 0, 2).reshape(D, HE)).astype(BF16NP),
            "wv": np.ascontiguousarray(
                W_V[h0:h0 + NHC].transpose(1, 0, 2).reshape(D, HE)).astype(BF16NP),
            "wo": np.ascontiguousarray(W_O[h0:h0 + NHC].reshape(HE, D)).astype(BF16NP),
            "mask": mask.astype(BF16NP),
        }
        if qkv_bias:
            im["bq"] = np.ascontiguousarray(b_Q[h0:h0 + NHC].reshape(HE, 1))
            im["bk"] = np.ascontiguousarray(b_K[h0:h0 + NHC].reshape(HE, 1))
            im["bv"] = np.ascontiguousarray(b_V[h0:h0 + NHC].reshape(1, HE)).astype(BF16NP)
        in_maps.append(im)

    import os
    trace = bool(os.environ.get("KERNEL_TRACE"))
    res = run_bass_kernel_spmd(nc, in_maps, core_ids=list(range(8)), trace=trace)
    LAST_EXEC_TIME_NS = res.exec_time_ns
    results = res.results

    out = np.empty((B, S, D), dtype=np.float32)
    for b in range(B):
        out[b] = results[2 * b]["out"] + results[2 * b + 1]["out"]
    if b_O.any():
        out += b_O
    return out
